# revision 1
# baseline (speedup 1.0000x reference)
"""Spectral-norm GRN kernel for trn2 (8 NeuronCores, batch-sharded SPMD).

out = gamma * (x * s) + beta + x,  s[b,c] = sigma_max(x[b,c]) / sum(sigma_max)

Per (b,c) 64x64 slice A:  G = (A^T A)/256, square 3x -> M8 = G^8 (PSUM).
sigma = 16 * (||M8||_F^2 / ||M4||_F^2)^(1/16)   [= tr(M^16)/tr(M^8) = lam^8]
Global sum of sigma via one AllReduce; output pass is x*scale+beta fused.
"""

import numpy as np

B, C, H, W = 16, 384, 64, 64
NCORES = 8
BPC = B // NCORES          # batches per core
S = BPC * C                # 768 slices per core
NG = S // 16               # 48 groups of 16 slices (8 pairs x 2 halves)
HALF = S // 2              # 384

_cache = {}


def _build():
    import concourse.bass as bass
    import concourse.bacc as bacc
    import concourse.mybir as mybir
    import concourse.tile as tile

    fp32 = mybir.dt.float32
    Act = mybir.ActivationFunctionType
    Alu = mybir.AluOpType

    nc = bacc.Bacc(None)
    x_t = nc.dram_tensor("x", [S, H, W], fp32, kind="ExternalInput")
    xp_t = nc.dram_tensor("xp", [128, NG, 512], fp32, kind="ExternalInput")
    g_t = nc.dram_tensor("g2", [128, 6], fp32, kind="ExternalInput")
    b_t = nc.dram_tensor("b2", [128, 6], fp32, kind="ExternalInput")
    y_t = nc.dram_tensor("y", [S, H, W], fp32, kind="ExternalOutput")

    ones_t = nc.inline_tensor(np.ones((128, 128), dtype=np.float32), "ones")
    ident_t = nc.inline_tensor(np.eye(128, dtype=np.float32), "ident")

    # src view for phase 2: [j, p, hw] with slice = 384*(j//3) + 128*(j%3) + p
    x_p2 = x_t[:].rearrange("(h k p) a b -> (h k) p (a b)", h=2, k=3)
    y_p2 = y_t[:].rearrange("(h k p) a b -> (h k) p (a b)", h=2, k=3)

    with tile.TileContext(nc) as tc:
        with (
            tc.tile_pool(name="sb", bufs=2) as sb,
            tc.tile_pool(name="xp", bufs=NG) as xpool,
            tc.tile_pool(name="sbg", bufs=2) as sbg,
            tc.tile_pool(name="one", bufs=1) as one,
            tc.tile_pool(name="ps", bufs=2, space="PSUM") as ps,
            tc.tile_pool(name="dram", bufs=1, space="DRAM") as dram,
        ):
            ones_sb = one.tile([128, 128], fp32, tag="ones")
            ident_sb = one.tile([128, 128], fp32, tag="ident")
            nc.sync.dma_start(ones_sb[:], ones_t[:])
            nc.sync.dma_start(ident_sb[:], ident_t[:])
            statD = one.tile([128, 384], fp32, tag="statD")
            statP = one.tile([128, 384], fp32, tag="statP")
            gT = one.tile([128, 6], fp32, tag="gT")
            bT = one.tile([128, 6], fp32, tag="bT")
            nc.sync.dma_start(gT[:], g_t[:])
            nc.sync.dma_start(bT[:], b_t[:])

            def mm_16(psum, src, start_col=0):
                # 16 matmuls: 8 q-blocks x 2 halves, quadrant-tiled
                for q in range(8):
                    for h in range(2):
                        p0 = h * 64
                        blk = src[p0:p0 + 64, q * 64:(q + 1) * 64]
                        out = psum[p0:p0 + 64, q * 64:(q + 1) * 64]
                        nc.tensor.matmul(out, blk, blk, start=True, stop=True,
                                         tile_position=(p0, p0))

            for g in range(NG):
                xT = xpool.tile([128, 512], fp32, tag="xT")
                nc.sync.dma_start(xT[:], xp_t[:, g, :])
                pG = ps.tile([128, 512], fp32, tag="pG")
                mm_16(pG, xT)
                G1 = sbg.tile([128, 512], fp32, tag="G1")
                nc.scalar.activation(G1[:], pG[:], Act.Copy, scale=1.0 / 256.0)
                pS1 = ps.tile([128, 512], fp32, tag="pS1")
                mm_16(pS1, G1)
                G2 = sbg.tile([128, 512], fp32, tag="G2")
                nc.vector.tensor_copy(G2[:], pS1[:])
                pS2 = ps.tile([128, 512], fp32, tag="pS2")
                mm_16(pS2, G2)
                G4 = sbg.tile([128, 512], fp32, tag="G4")
                nc.scalar.activation(G4[:], pS2[:], Act.Copy)
                pS3 = ps.tile([128, 512], fp32, tag="pS3")
                mm_16(pS3, G4)
                # stats: ||G4||^2 and ||G8||^2 row-partials per q-block
                sqA = sbg.tile([128, 512], fp32, tag="sqA")
                nc.gpsimd.tensor_tensor(sqA[:], G4[:], G4[:], Alu.mult)
                sqB = sbg.tile([128, 512], fp32, tag="sqB")
                nc.scalar.activation(sqB[:], pS3[:], Act.Square)
                nc.vector.tensor_reduce(
                    statD[:, g * 8:(g + 1) * 8],
                    sqA[:].rearrange("p (q w) -> p q w", q=8),
                    mybir.AxisListType.X, Alu.add)
                nc.vector.tensor_reduce(
                    statP[:, g * 8:(g + 1) * 8],
                    sqB[:].rearrange("p (q w) -> p q w", q=8),
                    mybir.AxisListType.X, Alu.add)

            # partition-reduce stats via PE transpose; trD/trP land in
            # phase-2 layout: col j=h*3+k holds slices 384h+128k+p
            trD = one.tile([128, 6], fp32, tag="trD")
            trP = one.tile([128, 6], fp32, tag="trP")
            for name, stat, dst in (("d", statD, trD), ("p", statP, trP)):
                for k in range(3):
                    pT = ps.tile([128, 128], fp32, tag="pG")
                    nc.tensor.transpose(pT[:], stat[:, k * 128:(k + 1) * 128],
                                        ident_sb[:])
                    nc.vector.tensor_reduce(
                        dst[:].rearrange("p (h k) -> p h k", h=2)[:, :, k],
                        pT[:].rearrange("p (h w) -> p h w", h=2),
                        mybir.AxisListType.X, Alu.add)

            # sigma = 16 * (trP/trD)^(1/16) = exp(ln(ratio)/16 + ln 16)
            zb = one.tile([128, 1], fp32, tag="zb")
            nc.vector.memset(zb[:], 0.0)
            rec = one.tile([128, 6], fp32, tag="rec")
            nc.vector.reciprocal(rec[:], trD[:])
            ratio = one.tile([128, 6], fp32, tag="ratio")
            nc.vector.tensor_tensor(ratio[:], trP[:], rec[:], Alu.mult)
            lnr = one.tile([128, 6], fp32, tag="lnr")
            nc.scalar.activation(lnr[:], ratio[:], Act.Ln, bias=zb[:, 0:1])
            sig = one.tile([128, 6], fp32, tag="sig")
            nc.scalar.activation(sig[:], lnr[:], Act.Exp,
                                 scale=1.0 / 16.0, bias=zb[:, 0:1])
            nc.vector.tensor_scalar_mul(sig[:], sig[:], 16.0)

            # local sum over 768 slices -> broadcast via ones-matmul
            srow = one.tile([128, 1], fp32, tag="srow")
            nc.vector.tensor_reduce(srow[:], sig[:], mybir.AxisListType.X,
                                    Alu.add)
            pSum = ps.tile([128, 1], fp32, tag="pG")
            nc.tensor.matmul(pSum[:], ones_sb[:], srow[:], start=True,
                             stop=True)
            locS = one.tile([128, 1], fp32, tag="locS")
            nc.vector.tensor_copy(locS[:], pSum[:])

            cc_in = dram.tile([128, 1], fp32)
            cc_out = dram.tile([128, 1], fp32)
            nc.sync.dma_start(cc_in[:], locS[:])
            nc.gpsimd.collective_compute(
                "AllReduce", Alu.add,
                replica_groups=[list(range(NCORES))],
                ins=[cc_in.opt()], outs=[cc_out.opt()])
            gS = one.tile([128, 1], fp32, tag="gS")
            nc.sync.dma_start(gS[:], cc_out[:])

            recS = one.tile([128, 1], fp32, tag="recS")
            nc.vector.reciprocal(recS[:], gS[:])
            # scale = 1 + gamma*sigma/S
            gsig = one.tile([128, 6], fp32, tag="gsig")
            nc.vector.tensor_tensor(gsig[:], gT[:], sig[:], Alu.mult)
            scaleT = one.tile([128, 6], fp32, tag="scaleT")
            nc.vector.tensor_scalar(scaleT[:], gsig[:], recS[:, 0:1], 1.0,
                                    Alu.mult, Alu.add)

            for j in range(6):
                X2 = sb.tile([128, 4096], fp32, tag="X2")
                nc.sync.dma_start(X2[:], x_p2[j])
                O2 = sb.tile([128, 4096], fp32, tag="O2")
                nc.vector.tensor_scalar(O2[:], X2[:], scaleT[:, j:j + 1],
                                        bT[:, j:j + 1], Alu.mult, Alu.add)
                nc.sync.dma_start(y_p2[j], O2[:])
    if not nc.is_finalized():
        nc.finalize()
    return nc


def _reorder(v):
    # [768] -> [128, 6] with v2[p, h*3+k] = v[384h + 128k + p]
    return np.ascontiguousarray(
        v.reshape(2, 3, 128).transpose(2, 0, 1).reshape(128, 6))


def _launch(x, gamma, beta, trace=False):
    from concourse.bass_utils import run_bass_kernel_spmd
    if "nc" not in _cache:
        _cache["nc"] = _build()
    nc = _cache["nc"]
    in_maps = []
    for c in range(NCORES):
        xl = np.ascontiguousarray(
            x[c * BPC:(c + 1) * BPC].reshape(S, H, W), dtype=np.float32)
        # phase-1 layout: xp[a*64+h, g, q*64+w] = xl[384a + 8g + q, h, w]
        xp = np.ascontiguousarray(
            xl.reshape(2, NG, 8, H, W).transpose(0, 3, 1, 2, 4)
            .reshape(128, NG, 512))
        gl = _reorder(gamma[c * BPC:(c + 1) * BPC].reshape(S).astype(np.float32))
        bl = _reorder(beta[c * BPC:(c + 1) * BPC].reshape(S).astype(np.float32))
        in_maps.append({"x": xl, "xp": xp, "g2": gl, "b2": bl})
    res = run_bass_kernel_spmd(nc, in_maps, core_ids=list(range(NCORES)),
                               trace=trace)
    out = np.empty((B, C, H, W), dtype=np.float32)
    for c in range(NCORES):
        out[c * BPC:(c + 1) * BPC] = res.results[c]["y"].reshape(BPC, C, H, W)
    return out, res


def kernel(x, gamma, beta):
    out, _ = _launch(np.asarray(x), np.asarray(gamma), np.asarray(beta))
    return out



# revision 4
# speedup vs baseline: 1.6426x; 1.6426x over previous
"""Spectral-norm GRN kernel for trn2 (8 NeuronCores, batch-sharded SPMD).

out = gamma * (x * s) + beta + x,  s[b,c] = sigma_max(x[b,c]) / sum(sigma_max)

Per (b,c) 64x64 slice A (bf16): G = (A^T A)/256, one squaring -> G^2.
sigma = 16 * (tr(G^4)/tr(G^2))^(1/4) = 16 * (||G^2||_F^2/||G||_F^2)^(1/4).
Global sum of sigma via one AllReduce; output pass is a single fused
per-partition x*scale+beta on the natural-layout fp32 copy of x.
"""

import numpy as np
import ml_dtypes

B, C, H, W = 16, 384, 64, 64
NCORES = 8
BPC = B // NCORES          # batches per core
S = BPC * C                # 768 slices per core
NG = S // 16               # 48 groups of 16 slices (8 q-blocks x 2 halves)
XPW = NG * 512             # xp free width (24576)

_cache = {}


def _build():
    import concourse.bass as bass
    import concourse.bacc as bacc
    import concourse.mybir as mybir
    import concourse.tile as tile

    fp32 = mybir.dt.float32
    bf16 = mybir.dt.bfloat16
    Act = mybir.ActivationFunctionType
    Alu = mybir.AluOpType

    nc = bacc.Bacc(None)
    x_t = nc.dram_tensor("x", [S, H, W], fp32, kind="ExternalInput")
    xp_t = nc.dram_tensor("xp", [128, XPW], bf16, kind="ExternalInput")
    g_t = nc.dram_tensor("g2", [128, 6], fp32, kind="ExternalInput")
    b_t = nc.dram_tensor("b2", [128, 6], fp32, kind="ExternalInput")
    y_t = nc.dram_tensor("y", [S, H, W], fp32, kind="ExternalOutput")

    ones_t = nc.inline_tensor(np.ones((128, 128), dtype=np.float32), "ones")
    ident_t = nc.inline_tensor(
        np.eye(128).astype(ml_dtypes.bfloat16), "ident")

    # natural-layout view: [j][128, 4096], slice = 384*(j//3) + 128*(j%3) + p
    x_p2 = x_t[:].rearrange("(h k p) a b -> (h k) p (a b)", h=2, k=3)
    y_p2 = y_t[:].rearrange("(h k p) a b -> (h k) p (a b)", h=2, k=3)

    with tile.TileContext(nc) as tc:
        with (
            tc.tile_pool(name="one", bufs=1) as one,
            tc.tile_pool(name="gb", bufs=3) as gb,
            tc.tile_pool(name="sq", bufs=3) as sqp,
            tc.tile_pool(name="psG", bufs=2, space="PSUM") as psG,
            tc.tile_pool(name="psS", bufs=2, space="PSUM") as psS,
            tc.tile_pool(name="psT", bufs=2, space="PSUM") as psT,
            tc.tile_pool(name="dram", bufs=1, space="DRAM") as dram,
        ):
            ones_sb = one.tile([128, 128], fp32, tag="ones")
            ident_sb = one.tile([128, 128], bf16, tag="ident")
            nc.sync.dma_start(ones_sb[:], ones_t[:])
            nc.sync.dma_start(ident_sb[:], ident_t[:])
            gT = one.tile([128, 6], fp32, tag="gT")
            bT = one.tile([128, 6], fp32, tag="bT")
            nc.sync.dma_start(gT[:], g_t[:])
            nc.sync.dma_start(bT[:], b_t[:])

            # resident inputs: xp (bf16, stats layout) and x (fp32, natural)
            xpR = one.tile([128, XPW], bf16, tag="xpR")
            for i in range(6):
                nc.sync.dma_start(xpR[:, i * 4096:(i + 1) * 4096],
                                  xp_t[:, i * 4096:(i + 1) * 4096])
            xnR = one.tile([128, 6 * 4096], fp32, tag="xnR")
            for j in range(6):
                nc.sync.dma_start(xnR[:, j * 4096:(j + 1) * 4096], x_p2[j])

            statD = one.tile([128, NG * 8], bf16, tag="statD")
            statP = one.tile([128, NG * 8], bf16, tag="statP")

            def mm16(psum, src):
                # 16 matmuls: 8 q-blocks x 2 halves, quadrant-tiled
                for q in range(8):
                    for h in range(2):
                        p0 = h * 64
                        blk = src[p0:p0 + 64, q * 64:(q + 1) * 64]
                        out = psum[p0:p0 + 64, q * 64:(q + 1) * 64]
                        nc.tensor.matmul(out, blk, blk, start=True, stop=True,
                                         tile_position=(p0, p0))

            # software-pipelined stats loop (1 squaring):
            #  PE:  s1(g), sq(g-1)
            #  Act: copyG(g), sqB(g-1)
            #  DVE: sqA(g-1), redB(g-2), redA(g-1)
            Gbf = [None] * NG
            pS = [None] * NG
            sqB = [None] * NG
            sqA = [None] * NG
            with nc.allow_low_precision(reason="bf16 trace partials"):
                for g in range(NG + 1):
                    if g < NG:
                        pG = psG.tile([128, 512], fp32, tag="pG")
                        mm16(pG, xpR[:, g * 512:(g + 1) * 512])
                    if g >= 1:
                        gp = g - 1
                        pS[gp] = psS.tile([128, 512], fp32, name="pS", tag="pS")
                        mm16(pS[gp], Gbf[gp][:])
                    if g < NG:
                        Gbf[g] = gb.tile([128, 512], bf16, name="Gbf", tag="Gbf")
                        nc.scalar.activation(Gbf[g][:], pG[:], Act.Copy,
                                             scale=1.0 / 256.0)
                    if g >= 1:
                        gp = g - 1
                        sqB[gp] = sqp.tile([128, 512], bf16, name="sqB", tag="sqB")
                        nc.scalar.activation(sqB[gp][:], pS[gp][:], Act.Square)
                        sqA[gp] = sqp.tile([128, 512], bf16, name="sqA", tag="sqA")
                        nc.vector.tensor_tensor(sqA[gp][:], Gbf[gp][:],
                                                Gbf[gp][:], Alu.mult)
                    if g >= 2:
                        gp = g - 2
                        nc.vector.tensor_reduce(
                            statP[:, gp * 8:(gp + 1) * 8],
                            sqB[gp][:].rearrange("p (q w) -> p q w", q=8),
                            mybir.AxisListType.X, Alu.add)
                        sqB[gp] = None
                    if g >= 1:
                        gp = g - 1
                        nc.vector.tensor_reduce(
                            statD[:, gp * 8:(gp + 1) * 8],
                            sqA[gp][:].rearrange("p (q w) -> p q w", q=8),
                            mybir.AxisListType.X, Alu.add)
                gp = NG - 1
                nc.vector.tensor_reduce(
                    statP[:, gp * 8:(gp + 1) * 8],
                    sqB[gp][:].rearrange("p (q w) -> p q w", q=8),
                    mybir.AxisListType.X, Alu.add)

            # partition-reduce stats via PE transpose; trD/trP land in
            # phase-2 layout: col j=a*3+k holds slice 384a+128k+p
            trD = one.tile([128, 6], fp32, tag="trD")
            trP = one.tile([128, 6], fp32, tag="trP")
            for stat, dst in ((statD, trD), (statP, trP)):
                for k in range(3):
                    pT = psT.tile([128, 128], bf16, tag="pT")
                    nc.tensor.transpose(pT[:], stat[:, k * 128:(k + 1) * 128],
                                        ident_sb[:])
                    nc.vector.tensor_reduce(
                        dst[:].rearrange("p (a k) -> p a k", a=2)[:, :, k],
                        pT[:].rearrange("p (a h) -> p a h", a=2),
                        mybir.AxisListType.X, Alu.add)

            # sigma = 16 * (trP/trD)^(1/4) = exp(ln(ratio)/4 + ln 16)
            ln16 = one.tile([128, 1], fp32, tag="ln16")
            nc.vector.memset(ln16[:], 2.772588722239781)
            rec = one.tile([128, 6], fp32, tag="rec")
            nc.vector.reciprocal(rec[:], trD[:])
            ratio = one.tile([128, 6], fp32, tag="ratio")
            nc.vector.tensor_tensor(ratio[:], trP[:], rec[:], Alu.mult)
            lnr = one.tile([128, 6], fp32, tag="lnr")
            nc.scalar.activation(lnr[:], ratio[:], Act.Ln)
            sig = one.tile([128, 6], fp32, tag="sig")
            nc.scalar.activation(sig[:], lnr[:], Act.Exp,
                                 scale=0.25, bias=ln16[:, 0:1])

            # local sum over 768 slices -> broadcast via ones-matmul
            gsig = one.tile([128, 6], fp32, tag="gsig")
            nc.vector.tensor_tensor(gsig[:], gT[:], sig[:], Alu.mult)
            srow = one.tile([128, 1], fp32, tag="srow")
            nc.vector.tensor_reduce(srow[:], sig[:], mybir.AxisListType.X,
                                    Alu.add)
            pSum = psT.tile([128, 1], fp32, tag="pT")
            nc.tensor.matmul(pSum[:], ones_sb[:], srow[:], start=True,
                             stop=True)
            locS = one.tile([128, 1], fp32, tag="locS")
            nc.vector.tensor_copy(locS[:], pSum[:])

            cc_in = dram.tile([128, 1], fp32)
            cc_out = dram.tile([128, 1], fp32)
            nc.sync.dma_start(cc_in[:], locS[:])
            nc.gpsimd.collective_compute(
                "AllReduce", Alu.add,
                replica_groups=[list(range(NCORES))],
                ins=[cc_in.opt()], outs=[cc_out.opt()])
            gS = one.tile([128, 1], fp32, tag="gS")
            nc.sync.dma_start(gS[:], cc_out[:])

            recS = one.tile([128, 1], fp32, tag="recS")
            nc.vector.reciprocal(recS[:], gS[:])
            # scale = 1 + gamma*sigma/S
            scaleT = one.tile([128, 6], fp32, tag="scaleT")
            nc.vector.tensor_scalar(scaleT[:], gsig[:], recS[:, 0:1], 1.0,
                                    Alu.mult, Alu.add)

            # output pass: in-place y = x*scale + beta on xnR, then store
            for j in range(6):
                for h2 in range(2):
                    seg = xnR[:, j * 4096 + h2 * 2048:j * 4096 + (h2 + 1) * 2048]
                    if (2 * j + h2) % 2 == 0:
                        nc.vector.tensor_scalar(seg, seg, scaleT[:, j:j + 1],
                                                bT[:, j:j + 1], Alu.mult,
                                                Alu.add)
                    else:
                        nc.scalar.activation(seg, seg, Act.Identity,
                                             bias=bT[:, j:j + 1],
                                             scale=scaleT[:, j:j + 1])
                nc.sync.dma_start(y_p2[j], xnR[:, j * 4096:(j + 1) * 4096])
    if not nc.is_finalized():
        nc.finalize()
    return nc


def _reorder(v):
    # [768] -> [128, 6] with v2[p, a*3+k] = v[384a + 128k + p]
    return np.ascontiguousarray(
        v.reshape(2, 3, 128).transpose(2, 0, 1).reshape(128, 6))


def _launch(x, gamma, beta, trace=False):
    from concourse.bass_utils import run_bass_kernel_spmd
    if "nc" not in _cache:
        _cache["nc"] = _build()
    nc = _cache["nc"]
    in_maps = []
    for c in range(NCORES):
        xl = np.ascontiguousarray(
            x[c * BPC:(c + 1) * BPC].reshape(S, H, W), dtype=np.float32)
        # stats layout: xp[a*64+h, g*512 + q*64 + w] = xl[384a + 8g + q, h, w]
        xp = np.ascontiguousarray(
            xl.reshape(2, NG, 8, H, W).transpose(0, 3, 1, 2, 4)
            .reshape(128, XPW)).astype(ml_dtypes.bfloat16)
        gl = _reorder(gamma[c * BPC:(c + 1) * BPC].reshape(S).astype(np.float32))
        bl = _reorder(beta[c * BPC:(c + 1) * BPC].reshape(S).astype(np.float32))
        in_maps.append({"x": xl, "xp": xp, "g2": gl, "b2": bl})
    res = run_bass_kernel_spmd(nc, in_maps, core_ids=list(range(NCORES)),
                               trace=trace)
    out = np.empty((B, C, H, W), dtype=np.float32)
    for c in range(NCORES):
        out[c * BPC:(c + 1) * BPC] = res.results[c]["y"].reshape(BPC, C, H, W)
    return out, res


def kernel(x, gamma, beta):
    out, _ = _launch(np.asarray(x), np.asarray(gamma), np.asarray(beta))
    return out
